# revision 1
# baseline (speedup 1.0000x reference)
"""GCNConv forward on 8 Trainium2 NeuronCores.

out = D^{-1/2} @ A @ x @ W + bias,  A sparse (edge list), D = row-degree.

Strategy (per sharding hint): shard destination rows across the 8 cores;
edge lists bucketed (sorted) by destination row on the host; x replicated
per core so each core gathers its source rows from local HBM via SWDGE
indirect DMA (one gathered row per partition per instruction); the
segment-sum rides the TensorEngine as a sequence of matmuls against fixed
block one-hot matrices; the 128x128 weight is applied as a second matmul;
degree scaling and bias ride DVE/ACT.

kernel() accepts the FULL inputs and returns the FULL output.
"""

import numpy as np

N_EXP, E_EXP, FIN, FOUT = 100000, 1_600_000, 128, 128
NCORES = 8
P = 128


def _numpy_reference(x, edge_row, edge_col, edge_val, weight, bias):
    deg = np.zeros(x.shape[0], np.float64)
    np.add.at(deg, edge_row, edge_val.astype(np.float64))
    dinv = 1.0 / np.sqrt(deg)
    support = np.zeros((x.shape[0], x.shape[1]), np.float64)
    np.add.at(support, edge_row, edge_val[:, None] * x[edge_col].astype(np.float64))
    return (support * dinv[:, None] @ weight + bias).astype(x.dtype)


_BUILD_CACHE = {}


def _build(T, K, apply_val, apply_bias, n_src, timing=False):
    """Compile the SPMD bass kernel. T dest tiles of 128 rows, K slots/row.

    timing=True keeps the device work identical but routes the per-tile
    output DMAs to an internal DRAM scratch with only a tiny external
    output, so wall-clock timing is not polluted by device-to-host pulls.
    """
    import concourse.bacc as bacc
    import concourse.bass as bass
    import concourse.mybir as mybir
    import concourse.tile as tile

    key = (T, K, apply_val, apply_bias, n_src, timing)
    if key in _BUILD_CACHE:
        return _BUILD_CACHE[key]

    nc = bacc.Bacc("TRN2", target_bir_lowering=False, debug=False, num_devices=NCORES)
    x = nc.declare_dram_parameter("x", [n_src, FIN], mybir.dt.float32, isOutput=False)
    idx = nc.declare_dram_parameter("idx", [T, P, K], mybir.dt.int32, isOutput=False)
    vgrid = nc.declare_dram_parameter("vgrid", [T, P, K], mybir.dt.float32, isOutput=False)
    onehots = nc.declare_dram_parameter("onehots", [P, K * P], mybir.dt.float32, isOutput=False)
    w = nc.declare_dram_parameter("w", [FIN, FOUT], mybir.dt.float32, isOutput=False)
    if apply_val:
        vbatch = nc.declare_dram_parameter("vbatch", [T, P, K], mybir.dt.float32, isOutput=False)
    if apply_bias:
        biasb = nc.declare_dram_parameter("biasb", [P, FOUT], mybir.dt.float32, isOutput=False)
    if timing:
        out = nc.dram_tensor("scratch", [T, P, FOUT], mybir.dt.float32)
        tiny = nc.declare_dram_parameter("tiny", [P, 1], mybir.dt.float32, isOutput=True)
    else:
        out = nc.declare_dram_parameter("out", [T, P, FOUT], mybir.dt.float32, isOutput=True)

    with tile.TileContext(nc) as tc:
        with (
            tc.tile_pool(name="const", bufs=1) as const_pool,
            tc.tile_pool(name="msgs", bufs=4) as msgs_pool,
            tc.tile_pool(name="idxp", bufs=3) as idx_pool,
            tc.tile_pool(name="vgp", bufs=3) as vg_pool,
            tc.tile_pool(name="sup", bufs=2) as sup_pool,
            tc.tile_pool(name="outp", bufs=3) as out_pool,
            tc.tile_pool(name="deg", bufs=2) as deg_pool,
            tc.tile_pool(name="ps", bufs=2, space="PSUM") as psum_pool,
            tc.tile_pool(name="ps2", bufs=2, space="PSUM") as psum2_pool,
        ):
            oh_sb = const_pool.tile([P, K * P], mybir.dt.float32)
            nc.sync.dma_start(out=oh_sb[:], in_=onehots[:])
            w_sb = const_pool.tile([FIN, FOUT], mybir.dt.float32)
            nc.sync.dma_start(out=w_sb[:], in_=w[:])
            if apply_bias:
                bias_sb = const_pool.tile([P, FOUT], mybir.dt.float32)
                nc.sync.dma_start(out=bias_sb[:], in_=biasb[:])

            for t in range(T):
                idx_t = idx_pool.tile([P, K], mybir.dt.int32)
                nc.sync.dma_start(out=idx_t[:], in_=idx[t])
                vg_t = vg_pool.tile([P, K], mybir.dt.float32)
                nc.sync.dma_start(out=vg_t[:], in_=vgrid[t])
                if apply_val:
                    vb_t = vg_pool.tile([P, K], mybir.dt.float32)
                    nc.sync.dma_start(out=vb_t[:], in_=vbatch[t])

                msgs = msgs_pool.tile([P, K, FIN], mybir.dt.float32)
                for s in range(K):
                    nc.gpsimd.indirect_dma_start(
                        out=msgs[:, s, :],
                        out_offset=None,
                        in_=x[:],
                        in_offset=bass.IndirectOffsetOnAxis(
                            ap=idx_t[:, s : s + 1], axis=0
                        ),
                    )
                if apply_val:
                    for s in range(K):
                        nc.vector.tensor_scalar_mul(
                            msgs[:, s, :], msgs[:, s, :], vb_t[:, s : s + 1]
                        )

                supT_ps = psum_pool.tile([FIN, P], mybir.dt.float32, space="PSUM")
                for s in range(K):
                    nc.tensor.matmul(
                        out=supT_ps[:],
                        lhsT=msgs[:, s, :],
                        rhs=oh_sb[:, s * P : (s + 1) * P],
                        start=(s == 0),
                        stop=(s == K - 1),
                    )
                supT_sb = sup_pool.tile([FIN, P], mybir.dt.float32)
                nc.vector.tensor_copy(supT_sb[:], supT_ps[:])

                # deg^{-1/2} per dest row
                deg_t = deg_pool.tile([P, 1], mybir.dt.float32)
                nc.vector.tensor_reduce(
                    out=deg_t[:], in_=vg_t[:],
                    axis=mybir.AxisListType.X, op=mybir.AluOpType.add,
                )
                dsq = deg_pool.tile([P, 1], mybir.dt.float32)
                nc.scalar.sqrt(dsq[:], deg_t[:])
                dinv = deg_pool.tile([P, 1], mybir.dt.float32)
                nc.vector.reciprocal(dinv[:], dsq[:])

                out_ps = psum2_pool.tile([P, FOUT], mybir.dt.float32, space="PSUM")
                nc.tensor.matmul(
                    out=out_ps[:], lhsT=supT_sb[:], rhs=w_sb[:],
                    start=True, stop=True,
                )
                out_sb = out_pool.tile([P, FOUT], mybir.dt.float32)
                nc.vector.tensor_scalar_mul(out_sb[:], out_ps[:], dinv[:, 0:1])
                if apply_bias:
                    nc.vector.tensor_tensor(
                        out=out_sb[:], in0=out_sb[:], in1=bias_sb[:],
                        op=mybir.AluOpType.add,
                    )
                nc.sync.dma_start(out=out[t], in_=out_sb[:])
                if timing and t == T - 1:
                    nc.sync.dma_start(out=tiny[:], in_=out_sb[:, 0:1])
    nc.compile()
    _BUILD_CACHE[key] = nc
    return nc


def _prepare(x, edge_row, edge_col, edge_val, weight, bias):
    """Host-side bucketing/sharding. Returns (meta, in_maps)."""
    N = x.shape[0]
    E = edge_row.shape[0]

    order = np.argsort(edge_row, kind="stable")
    row_s = edge_row[order]
    col_s = edge_col[order]
    val_s = edge_val[order]

    counts = np.bincount(edge_row, minlength=N)
    max_deg = int(counts.max()) if E else 1
    uniform = bool((counts == max_deg).all())
    ones = bool(np.all(edge_val == 1.0))

    # pick K (slots per row): must divide 128
    K = 1
    while K < max_deg:
        K *= 2
    if K > 128:
        return None  # fallback to numpy path
    fast = uniform and ones and max_deg == K

    R_core = -(-N // (NCORES * P)) * P  # dest rows per core, tile-padded
    T = R_core // P
    N_pad = R_core * NCORES

    if fast:
        src_pad = col_s.reshape(N, K)
        val_pad = val_s.reshape(N, K).astype(np.float32)
    else:
        src_pad = np.zeros((N, K), np.int32)
        val_pad = np.zeros((N, K), np.float32)
        pos = np.arange(E) - np.repeat(np.cumsum(counts) - counts, counts)
        src_pad[row_s, pos] = col_s
        val_pad[row_s, pos] = val_s
    if N_pad > N:
        src_pad = np.concatenate([src_pad, np.zeros((N_pad - N, K), src_pad.dtype)])
        val_pad = np.concatenate([val_pad, np.zeros((N_pad - N, K), np.float32)])

    # fixed block one-hot matrices: onehot_s[e, d] = 1 iff d == s*(128//K) + e//K
    e_ar = np.arange(P)
    oh = np.zeros((P, K * P), np.float32)
    for s in range(K):
        oh[e_ar, s * P + s * (P // K) + e_ar // K] = 1.0

    apply_bias = bool(np.any(bias != 0.0))
    biasb = np.tile(bias.astype(np.float32)[None, :], (P, 1))

    in_maps = []
    for c in range(NCORES):
        sl = slice(c * R_core, (c + 1) * R_core)
        src_c = src_pad[sl]  # [R_core, K] dest-major
        val_c = val_pad[sl]
        # edge-major grids: [T, P, K] with [t, p, s] = edge (t, s*128+p)
        # edge j of tile t (sorted) = (dest j//K, slot j%K) ->
        # entry for (p, s) is dest-major element [t, (s*128+p)//K, (s*128+p)%K]
        seq_src = src_c.reshape(T, P * K)  # [t, j] j = dest-major edge index
        seq_val = val_c.reshape(T, P * K)
        # mapping grid [P, K]: batch-s edge p sits at dest-major index s*128 + p
        jj = np.arange(P)[:, None] + np.arange(K)[None, :] * P  # [p, s]
        idx_g = seq_src[:, jj.reshape(-1)].reshape(T, P, K).astype(np.int32)
        vb_g = seq_val[:, jj.reshape(-1)].reshape(T, P, K).astype(np.float32)
        vg_g = val_c.reshape(T, P, K).astype(np.float32)
        m = {
            "x": np.ascontiguousarray(x.astype(np.float32)),
            "idx": np.ascontiguousarray(idx_g),
            "vgrid": np.ascontiguousarray(vg_g),
            "onehots": oh,
            "w": np.ascontiguousarray(weight.astype(np.float32)),
        }
        if not fast:
            m["vbatch"] = np.ascontiguousarray(vb_g)
        if apply_bias:
            m["biasb"] = biasb
        in_maps.append(m)
    meta = dict(T=T, K=K, fast=fast, apply_bias=apply_bias, N=N, R_core=R_core,
                n_src=x.shape[0])
    return meta, in_maps


def kernel(x, edge_row, edge_col, edge_val, weight, bias):
    x = np.asarray(x)
    edge_row = np.asarray(edge_row)
    edge_col = np.asarray(edge_col)
    edge_val = np.asarray(edge_val)
    weight = np.asarray(weight)
    bias = np.asarray(bias)

    prep = _prepare(x, edge_row, edge_col, edge_val, weight, bias)
    if prep is None:
        return _numpy_reference(x, edge_row, edge_col, edge_val, weight, bias)
    meta, in_maps = prep

    from concourse.bass_utils import run_bass_kernel_spmd

    nc = _build(meta["T"], meta["K"], not meta["fast"], meta["apply_bias"],
                meta["n_src"])
    res = run_bass_kernel_spmd(nc, in_maps, list(range(NCORES)))
    outs = [res.results[c]["out"].reshape(meta["R_core"], FOUT)
            for c in range(NCORES)]
    full = np.concatenate(outs, axis=0)[: meta["N"]]
    return full.astype(x.dtype)

